# revision 1
# baseline (speedup 1.0000x reference)
"""Trainium2 Bass kernel: causal self-attention with RoPE (B=4, T=2048, D=1024, H=16, Dh=64).

Sharding: 8 cores = 4 batches x 2 head-halves. Core c handles batch c//2 and
heads (c%2)*8 .. (c%2)*8+7 (feature columns (c%2)*512 .. +512 of Wq/Wk/Wv, and
the matching rows of Wo). Each core computes a partial output [T, D]; the host
sums the two partials per batch (row-sharded Wo reduction) and stacks batches.

On-chip layout: activations are kept transposed (features on partitions):
  xT [D, T] (spilled to DRAM), qT/kT [512, T], scoresT [s, t], attn_outT [512, T].
This makes every matmul contraction land on the partition dim with zero
transposes except one PE-transpose pass over x. The softmax denominator is
fused into the AV matmul via a ones-column appended to V (M=65), and the
causal mask is applied post-exp with a single tensor_mask per diagonal group.
"""

import os
import sys

for _p in ("/opt/trn_rl_repo", "/root/.axon_site/_ro/trn_rl_repo"):
    if os.path.isdir(_p) and _p not in sys.path:
        sys.path.append(_p)

import numpy as np

import bass_rust
import concourse.bass as bass
import concourse.mybir as mybir
import concourse.tile as tile
from concourse.vector_clock import ScopedClock

F32 = mybir.dt.float32
F32R = mybir.dt.float32r
BF16 = mybir.dt.bfloat16

B, T, D, H, Dh = 4, 2048, 1024, 16, 64
FC = 512          # features per core (8 heads)
NG = 2            # head groups per core (4 heads each)
FG = FC // NG     # 256 features per group
NTC = T // 512    # 4 t-chunks
NTT = T // 128    # 16 t-tiles
ND = D // 128     # 8 d-chunks


class _TC(tile.TileContext):
    """TileContext whose tail Drain carries at most one sem wait.

    The walrus build in this container rejects a Drain with >1 sync waits
    (setupSyncWait: "Too many sync wait commands"), so spread the waits over
    a chain of Drain instructions instead.
    """

    def _drain_and_barrier(self, tick_clock, wait_clock):
        drain_inst = self.nc.sync.drain()
        wait_clock.add_sem_waits(
            drain_inst.ins, ScopedClock({None: tick_clock.global_clock})
        )
        si = drain_inst.ins.sync_info
        if si is not None and len(si.on_wait) > 1:
            waits = list(si.on_wait)
            drain_inst.ins.sync_info = bass_rust.SyncInfo(
                on_wait=waits[:1], on_update=list(si.on_update)
            )
            for w in waits[1:]:
                d2 = self.nc.sync.drain()
                d2.ins.sync_info = bass_rust.SyncInfo(on_wait=[w], on_update=[])
        self.nc.all_engine_barrier()
        popped = self.nc._tile_sem_poison_stack.pop()
        assert popped is self._sem_poison
        self.nc.clear_and_free_semaphores(list(self.sems.allocated().values()))
        self.nc.all_engine_barrier()


def _r(ap):
    return ap.bitcast(F32R)


def _split_waits(nc, max_waits=1):
    """Hoist extra sem waits onto same-engine NoOps.

    The walrus build here allows only one sync wait on several instruction
    structs (Drain, the fp32/fp32r matmul LW struct). Engine queues are
    in-order, so moving waits to a preceding NoOp on the same engine is
    semantics-preserving.
    """
    n = 0
    for fn in nc.m.functions:
        for bb in fn.blocks:
            out = []
            for inst in bb.instructions:
                si = inst.sync_info
                if si is not None and len(si.on_wait) > max_waits:
                    waits = list(si.on_wait)
                    extra, keep = waits[:-max_waits], waits[-max_waits:]
                    for i, w in enumerate(extra):
                        nop = mybir.InstNoOp(
                            name=f"{inst.name}_ws{i}", engine=inst.engine
                        )
                        nop.sync_info = bass_rust.SyncInfo(on_wait=[w], on_update=[])
                        out.append(nop)
                        n += 1
                    inst.sync_info = bass_rust.SyncInfo(
                        on_wait=keep, on_update=list(si.on_update)
                    )
                out.append(inst)
            bb.instructions = out
    return n


def _build_program():
    from contextlib import ExitStack

    nc = bass.Bass()

    x = nc.dram_tensor("x", [T, D], F32, kind="ExternalInput")
    wq = nc.dram_tensor("wq", [D, FC], F32R, kind="ExternalInput")
    wk = nc.dram_tensor("wk", [D, FC], F32R, kind="ExternalInput")
    wv = nc.dram_tensor("wv", [D, FC], F32R, kind="ExternalInput")
    wo = nc.dram_tensor("wo", [FC, D], F32R, kind="ExternalInput")
    cos2 = nc.dram_tensor("cos2", [128, T], F32, kind="ExternalInput")
    sin2 = nc.dram_tensor("sin2", [128, T], F32, kind="ExternalInput")
    ident = nc.dram_tensor("ident", [128, 128], F32, kind="ExternalInput")
    mk0 = nc.dram_tensor("mk0", [128, 1024], F32, kind="ExternalInput")
    mk256 = nc.dram_tensor("mk256", [128, 1024], F32, kind="ExternalInput")
    ones8 = nc.dram_tensor("ones8", [128, 8], F32R, kind="ExternalInput")
    ones64 = nc.dram_tensor("ones64", [1, 64], F32R, kind="ExternalInput")
    out = nc.dram_tensor("out", [T, D], F32, kind="ExternalOutput")

    with _TC(nc) as tc, ExitStack() as ctx:
        consts = ctx.enter_context(tc.tile_pool(name="consts", bufs=1))
        # PSUM: 2x [128,1024] double-bank slots + 4x [128,512] single-bank slots
        psum = ctx.enter_context(tc.tile_pool(name="psum", bufs=2, space="PSUM"))
        psums = ctx.enter_context(tc.tile_pool(name="psums", bufs=4, space="PSUM"))
        dram = ctx.enter_context(tc.tile_pool(name="dram", bufs=4, space="DRAM"))
        persist = ctx.enter_context(tc.tile_pool(name="persist", bufs=1))
        wp = ctx.enter_context(tc.tile_pool(name="wp", bufs=1))

        ident_t = consts.tile([128, 128], F32)
        nc.sync.dma_start(ident_t[:], ident[:])
        ones64_t = consts.tile([1, 64], F32R)
        nc.sync.dma_start(ones64_t[:], ones64[:])
        mk0_t = consts.tile([128, 1024], F32)
        nc.sync.dma_start(mk0_t[:], mk0[:])
        mk256_t = consts.tile([128, 1024], F32)
        nc.sync.dma_start(mk256_t[:], mk256[:])

        def load_weights(g):
            gsl = slice(g * FG, (g + 1) * FG)
            tiles = []
            for nm, wsrc in (("wq", wq), ("wk", wk)):
                w_t = wp.tile([128, ND * FG], F32R, tag=nm, name=f"{nm}_t{g}")
                nc.sync.dma_start(
                    w_t[:].rearrange("p (d f) -> p d f", d=ND),
                    wsrc[:, gsl].rearrange("(d p) f -> p d f", p=128),
                )
                tiles.append(w_t)
            return tiles

        wv_t = wp.tile([128, ND * FC], F32R, tag="wv", name="wv_t")
        nc.sync.dma_start(
            wv_t[:].rearrange("p (d f) -> p d f", d=ND),
            wv[:].rearrange("(d p) f -> p d f", p=128),
        )

        # attention outputs, persistent across both groups: 4 f-chunks [128, T]
        ao = [persist.tile([128, T], F32R, tag=f"ao{i}", name=f"ao{i}") for i in range(4)]
        # v (natural layout) with a ones column per head: 8 heads x 65 cols
        vt = [persist.tile([128, 8 * 65], F32R, tag=f"vt{i}", name=f"vt{i}") for i in range(NTT)]

        wtiles = load_weights(0)
        for i in range(NTT):
            nc.sync.dma_start(vt[i][:, 64::65], ones8[:])
        for g in range(NG):
            # per-group persistent activations (slots reused across groups)
            qt = [persist.tile([128, T], BF16, tag=f"qt{i}", name=f"qt{i}g{g}") for i in range(2)]
            kt = [persist.tile([128, T], BF16, tag=f"kt{i}", name=f"kt{i}g{g}") for i in range(2)]

            with ExitStack() as gctx:
                # ---- Phase B(g): projections + RoPE
                xtc = gctx.enter_context(tc.tile_pool(name=f"xtc{g}", bufs=3))
                rop = gctx.enter_context(tc.tile_pool(name=f"rope{g}", bufs=3))
                xld = gctx.enter_context(tc.tile_pool(name=f"xload{g}", bufs=4))

                wq_t, wk_t = wtiles

                for tcc in range(NTC):
                    csl = slice(tcc * 512, (tcc + 1) * 512)
                    cos_t = rop.tile([128, 512], F32, tag="cs", name="cos_t", bufs=2)
                    nc.sync.dma_start(cos_t[:], cos2[:, csl])
                    sin_t = rop.tile([128, 512], F32, tag="sn", name="sin_t", bufs=2)
                    nc.sync.dma_start(sin_t[:], sin2[:, csl])
                    # rotate_half sign fold: rows 0:32 / 64:96 get -sin
                    nc.scalar.mul(sin_t[0:32, :], sin_t[0:32, :], -1.0)
                    nc.scalar.mul(sin_t[64:96, :], sin_t[64:96, :], -1.0)
                    xc = xtc.tile([128, ND * 512], F32R, tag="xc")
                    # transpose x[tc] on the fly, one x-tile at a time
                    xcv = xc[:].rearrange("p (d t) -> p d t", d=ND)
                    for q in range(4):
                        t0 = (tcc * 4 + q) * 128
                        xt_ = xld.tile([128, D], F32, tag="xl")
                        nc.sync.dma_start(xt_[:], x[t0 : t0 + 128, :])
                        for dh in range(2):
                            tp = psums.tile([128, 512], F32, tag="sm", name="tp")
                            for dl in range(4):
                                d = dh * 4 + dl
                                nc.tensor.transpose(
                                    tp[:, dl * 128 : (dl + 1) * 128],
                                    xt_[:, d * 128 : (d + 1) * 128],
                                    ident_t[:],
                                )
                            nc.vector.tensor_copy(
                                xcv[:, dh * 4 : dh * 4 + 4, q * 128 : (q + 1) * 128],
                                tp[:].rearrange("p (dl t) -> p dl t", dl=4),
                            )
                    tsl = slice(tcc * 512, (tcc + 1) * 512)
                    # q/k projections (transposed outputs) + RoPE
                    for dst, w_t in ((qt, wq_t), (kt, wk_t)):
                        ps = psum.tile([128, 1024], F32, tag="pp")
                        for fp in range(2):
                            for d in range(ND):
                                nc.tensor.matmul(
                                    ps[:, fp * 512 : fp * 512 + 512],
                                    w_t[:, d * FG + fp * 128 : d * FG + (fp + 1) * 128],
                                    xc[:, d * 512 : (d + 1) * 512],
                                    start=(d == 0),
                                    stop=(d == ND - 1),
                                )
                        for fp in range(2):
                            psl = ps[:, fp * 512 : fp * 512 + 512]
                            raw = rop.tile([128, 512], F32, tag="raw")
                            nc.scalar.copy(raw[:], psl)
                            rot = rop.tile([128, 512], F32, tag="rot")
                            for hb in range(2):
                                o = hb * 64
                                nc.sync.dma_start(rot[o : o + 32, :], raw[o + 32 : o + 64, :])
                                nc.sync.dma_start(rot[o + 32 : o + 64, :], raw[o : o + 32, :])
                            dtile = dst[fp]
                            nc.vector.tensor_mul(dtile[:, tsl], psl, cos_t[:])
                            nc.vector.tensor_mul(rot[:], rot[:], sin_t[:])
                            nc.vector.tensor_add(dtile[:, tsl], dtile[:, tsl], rot[:])
                    # v projection for all 8 heads at once (group 0 only)
                    if g == 0:
                        for tb in range(4):
                            pv = psums.tile([128, 512], F32, tag="sm", name="pv")
                            for d in range(ND):
                                nc.tensor.matmul(
                                    pv[:],
                                    xc[:, d * 512 + tb * 128 : d * 512 + (tb + 1) * 128],
                                    wv_t[:, d * FC : (d + 1) * FC],
                                    start=(d == 0),
                                    stop=(d == ND - 1),
                                )
                            i = tcc * 4 + tb
                            vdst = vt[i][:, :].rearrange("p (h c) -> p h c", c=65)[
                                :, :, 0:64
                            ]
                            vsrc = pv[:].rearrange("p (h c) -> p h c", c=64)
                            nc.vector.tensor_copy(vdst, vsrc)

                if g == 0:
                    # prefetch group-1 weights while attention of group 0 runs
                    wtiles = load_weights(1)

            # ---- Phase C(g): attention. The pair's two heads are interleaved
            # so one head's QK fills the PE while the other waits on exp.
            with ExitStack() as cctx:
                ep = cctx.enter_context(tc.tile_pool(name=f"exp{g}", bufs=4))
                rp = cctx.enter_context(tc.tile_pool(name=f"rcp{g}", bufs=3))
                for tcc in (3, 2, 1, 0):
                    tsl = slice(tcc * 512, (tcc + 1) * 512)
                    ngrp = 2 * tcc + 2
                    for fp in range(2):
                        avs = []
                        for ho in range(2):
                            av_ps = psums.tile(
                                [128, 512], F32, tag="sm", name=f"av{ho}"
                            )
                            avs.append(av_ps)
                        for g2 in range(ngrp):
                            exs = []
                            for ho in range(2):
                                o = ho * 64
                                sc = psum.tile([128, 1024], F32, tag="pp", name="sc")
                                for half in range(2):
                                    si = 2 * g2 + half
                                    nc.tensor.matmul(
                                        sc[:, half * 512 : half * 512 + 512],
                                        kt[fp][o : o + 64, si * 128 : (si + 1) * 128],
                                        qt[fp][o : o + 64, tsl],
                                        start=True,
                                        stop=True,
                                    )
                                ex = ep.tile([128, 1024], F32R, tag="ex")
                                nc.scalar.activation(
                                    ex[:], sc[:], mybir.ActivationFunctionType.Exp,
                                    scale=0.125,
                                )
                                if g2 >= 2 * tcc:
                                    mt = mk0_t if g2 == 2 * tcc else mk256_t
                                    nc.vector.tensor_mul(ex[:], ex[:], mt[:])
                                exs.append(ex)
                            for ho in range(2):
                                hl = 2 * fp + ho
                                for half in range(2):
                                    si = 2 * g2 + half
                                    nc.tensor.matmul(
                                        avs[ho][0:65, :],
                                        vt[si][:, (4 * g + hl) * 65 : (4 * g + hl) * 65 + 65],
                                        exs[ho][:, half * 512 : half * 512 + 512],
                                        start=(g2 == 0 and half == 0),
                                        stop=(g2 == ngrp - 1 and half == 1),
                                    )
                        for ho in range(2):
                            o = ho * 64
                            av_sb = rp.tile([65, 512], F32, tag="avs")
                            nc.vector.tensor_copy(av_sb[:], avs[ho][0:65, :])
                            rcp = rp.tile([1, 512], F32R, tag="rc")
                            with nc.allow_low_precision(reason="f32r recip"):
                                nc.vector.reciprocal(rcp[:], av_sb[64:65, :])
                            pb = psums.tile([128, 512], F32, tag="sm", name="pb")
                            nc.tensor.matmul(
                                pb[0:64, :], ones64_t[:], rcp[:], start=True, stop=True
                            )
                            nc.vector.tensor_mul(
                                ao[2 * g + fp][o : o + 64, tsl],
                                av_sb[0:64, :],
                                pb[0:64, :],
                            )

        # ---- Phase D: output projection (row-sharded Wo partial)
        with tc.tile_pool(name="wo", bufs=1) as wop, tc.tile_pool(
            name="oev", bufs=3
        ) as oev:
            wo_t = wop.tile([128, 4 * D], F32R, tag="wo")
            nc.sync.dma_start(
                wo_t[:].rearrange("p (c o) -> p c o", c=4),
                wo[:].rearrange("(c p) o -> p c o", p=128),
            )
            for i in range(NTT):
                po = psum.tile([128, 1024], F32, tag="pp", name="po")
                for n in range(2):
                    for c in range(4):
                        nc.tensor.matmul(
                            po[:, n * 512 : n * 512 + 512],
                            ao[c][:, i * 128 : (i + 1) * 128],
                            wo_t[:, c * D + n * 512 : c * D + n * 512 + 512],
                            start=(c == 0),
                            stop=(c == 3),
                        )
                oe = oev.tile([128, 1024], F32, tag="oe")
                nc.scalar.copy(oe[:], po[:])
                nc.sync.dma_start(out[i * 128 : (i + 1) * 128, :], oe[:])

    _split_waits(nc)
    return nc

_NC_CACHE = None


def _get_nc():
    global _NC_CACHE
    if _NC_CACHE is None:
        _NC_CACHE = _build_program()
    return _NC_CACHE


def _consts():
    j = np.arange(1024)
    p = np.arange(128)
    s_rel = p[:, None] + 128 * (j[None, :] // 512)  # s offset within group
    t_rel = j[None, :] % 512
    return {
        "ident": np.eye(128, dtype=np.float32),
        "mk0": (s_rel <= t_rel).astype(np.float32),
        "mk256": (s_rel + 256 <= t_rel).astype(np.float32),
        "ones8": np.ones((128, 8), dtype=np.float32),
        "ones64": np.ones((1, 64), dtype=np.float32),
    }


def kernel(x, cos, sin, Wq, Wk, Wv, Wo):
    from concourse.bass_utils import run_bass_kernel_spmd

    x = np.asarray(x, dtype=np.float32)
    cos = np.asarray(cos, dtype=np.float32)
    sin = np.asarray(sin, dtype=np.float32)
    Wq = np.asarray(Wq, dtype=np.float32)
    Wk = np.asarray(Wk, dtype=np.float32)
    Wv = np.asarray(Wv, dtype=np.float32)
    Wo = np.asarray(Wo, dtype=np.float32)

    cos2 = np.ascontiguousarray(np.tile(cos.T, (2, 1)))  # [128, T]
    sin2 = np.ascontiguousarray(np.tile(sin.T, (2, 1)))
    consts = _consts()

    in_maps = []
    for c in range(8):
        b, hh = c // 2, c % 2
        sl = slice(hh * FC, (hh + 1) * FC)
        in_maps.append(
            {
                "x": np.ascontiguousarray(x[b]),
                "wq": np.ascontiguousarray(Wq[:, sl]),
                "wk": np.ascontiguousarray(Wk[:, sl]),
                "wv": np.ascontiguousarray(Wv[:, sl]),
                "wo": np.ascontiguousarray(Wo[sl, :]),
                "cos2": cos2,
                "sin2": sin2,
                **consts,
            }
        )

    nc = _get_nc()
    res = run_bass_kernel_spmd(nc, in_maps, core_ids=list(range(8)))
    outs = [res.results[c]["out"] for c in range(8)]
    full = np.stack([outs[2 * b] + outs[2 * b + 1] for b in range(B)])
    return full.astype(np.float32)



# revision 5
# speedup vs baseline: 1.4017x; 1.4017x over previous
"""Trainium2 Bass kernel v2: causal self-attention with RoPE, all-fp16 PE.

B=4, T=2048, D=1024, H=16, Dh=64. 8 cores = 4 batches x 2 head-halves.
Core c: batch c//2, heads (c%2)*8..(c%2)*8+7. Host sums the two partials
per batch (row-sharded Wo reduction).

vs the f32r baseline:
- Host pre-transposes x (no on-chip PE transposes at all) and converts
  x/Wq/Wk/Wv/Wo/cos/sin to fp16; sin's rotate-half sign is pre-folded.
- Every matmul is fp16 (1 cycle/row; f32r only reaches that >=256 rows,
  and fp32 transposes cost 2x).
- Causal trimming: QK/AV matmuls skip fully-masked query ranges; the
  within-block diagonal triangle is masked by one [128,128] fp16 mul.
- exp is the only Activation-engine work (no table thrash, no copies);
  RoPE runs on DVE in fp16 (2x mode); softmax denominator reciprocal is
  broadcast by gpsimd.partition_broadcast (no PE broadcast matmul).
- proj/attention/out-proj interleaved per t-chunk so PE/ACT/DVE overlap.
"""

import os
import sys

for _p in ("/opt/trn_rl_repo", "/root/.axon_site/_ro/trn_rl_repo"):
    if os.path.isdir(_p) and _p not in sys.path:
        sys.path.append(_p)

import numpy as np

import bass_rust
import concourse.bass as bass
import concourse.mybir as mybir
import concourse.tile as tile
from concourse.vector_clock import ScopedClock

F32 = mybir.dt.float32
F16 = mybir.dt.float16

B, T, D, H, Dh = 4, 2048, 1024, 16, 64
FC = 512          # features per core (8 heads)
NF = 4            # f-tiles of 128 (2 heads each)
NTC = T // 512    # 4 t-chunks
NTT = T // 128    # 16 t-tiles
ND = 8            # d-chunks of 128
EXP_SCALE = 0.125


class _TC(tile.TileContext):
    """TileContext whose tail Drain carries at most one sem wait.

    The walrus build in this container rejects a Drain with >1 sync waits
    (setupSyncWait: "Too many sync wait commands"), so spread the waits over
    a chain of Drain instructions instead.
    """

    def _drain_and_barrier(self, tick_clock, wait_clock):
        drain_inst = self.nc.sync.drain()
        wait_clock.add_sem_waits(
            drain_inst.ins, ScopedClock({None: tick_clock.global_clock})
        )
        si = drain_inst.ins.sync_info
        if si is not None and len(si.on_wait) > 1:
            waits = list(si.on_wait)
            drain_inst.ins.sync_info = bass_rust.SyncInfo(
                on_wait=waits[:1], on_update=list(si.on_update)
            )
            for w in waits[1:]:
                d2 = self.nc.sync.drain()
                d2.ins.sync_info = bass_rust.SyncInfo(on_wait=[w], on_update=[])
        self.nc.all_engine_barrier()
        popped = self.nc._tile_sem_poison_stack.pop()
        assert popped is self._sem_poison
        self.nc.clear_and_free_semaphores(list(self.sems.allocated().values()))
        self.nc.all_engine_barrier()


def _split_waits(nc, max_waits=1):
    """Hoist extra sem waits onto same-engine NoOps (walrus 1-wait limit)."""
    n = 0
    for fn in nc.m.functions:
        for bb in fn.blocks:
            out = []
            for inst in bb.instructions:
                si = inst.sync_info
                if si is not None and len(si.on_wait) > max_waits:
                    waits = list(si.on_wait)
                    extra, keep = waits[:-max_waits], waits[-max_waits:]
                    for i, w in enumerate(extra):
                        nop = mybir.InstNoOp(
                            name=f"{inst.name}_ws{i}", engine=inst.engine
                        )
                        nop.sync_info = bass_rust.SyncInfo(on_wait=[w], on_update=[])
                        out.append(nop)
                        n += 1
                    inst.sync_info = bass_rust.SyncInfo(
                        on_wait=keep, on_update=list(si.on_update)
                    )
                out.append(inst)
            bb.instructions = out
    return n


def _build_program():
    from contextlib import ExitStack

    nc = bass.Bass()

    xt16 = nc.dram_tensor("xt16", [128, 8 * T], F16, kind="ExternalInput")
    wq16 = nc.dram_tensor("wq16", [128, 8 * FC], F16, kind="ExternalInput")
    wk16 = nc.dram_tensor("wk16", [128, 8 * FC], F16, kind="ExternalInput")
    wv16 = nc.dram_tensor("wv16", [128, 8 * FC], F16, kind="ExternalInput")
    wo16 = nc.dram_tensor("wo16", [128, 4 * D], F16, kind="ExternalInput")
    cos2 = nc.dram_tensor("cos2", [128, T], F16, kind="ExternalInput")
    sin2 = nc.dram_tensor("sin2", [128, T], F16, kind="ExternalInput")
    tri = nc.dram_tensor("tri", [128, 128], F16, kind="ExternalInput")
    ones8 = nc.dram_tensor("ones8", [128, 8], F16, kind="ExternalInput")
    ones64 = nc.dram_tensor("ones64", [1, 64], F16, kind="ExternalInput")
    out = nc.dram_tensor("out", [T, D], F32, kind="ExternalOutput")

    with _TC(nc) as tc, ExitStack() as ctx:
        consts = ctx.enter_context(tc.tile_pool(name="consts", bufs=1))
        persist = ctx.enter_context(tc.tile_pool(name="persist", bufs=1))
        # PSUM budget (8 banks): ppp 2x[128,512] + psc 2x[128,1024] + pav 2x[128,512]
        ppp = ctx.enter_context(tc.tile_pool(name="ppp", bufs=2, space="PSUM"))
        psc = ctx.enter_context(tc.tile_pool(name="psc", bufs=2, space="PSUM"))
        pav = ctx.enter_context(tc.tile_pool(name="pav", bufs=2, space="PSUM"))
        rope = ctx.enter_context(tc.tile_pool(name="rope", bufs=3))
        exp = ctx.enter_context(tc.tile_pool(name="exp", bufs=4))
        nrm = ctx.enter_context(tc.tile_pool(name="nrm", bufs=2))
        oev = ctx.enter_context(tc.tile_pool(name="oev", bufs=3))

        cos_t = consts.tile([128, T], F16)
        nc.sync.dma_start(cos_t[:], cos2[:])
        sin_t = consts.tile([128, T], F16)
        nc.sync.dma_start(sin_t[:], sin2[:])
        tri_t = consts.tile([128, 128], F16)
        nc.sync.dma_start(tri_t[:], tri[:])
        ones8_t = consts.tile([128, 8], F16)
        nc.sync.dma_start(ones8_t[:], ones8[:])
        ones64_t = consts.tile([1, 64], F16)
        nc.sync.dma_start(ones64_t[:], ones64[:])

        w_ts = {}
        for nm, wsrc in (("wq", wq16), ("wk", wk16), ("wv", wv16)):
            w_t = persist.tile([128, 8 * FC], F16, tag=nm, name=nm)
            nc.sync.dma_start(w_t[:], wsrc[:])
            w_ts[nm] = w_t
        wo_t = persist.tile([128, 4 * D], F16, tag="wo", name="wo")
        nc.sync.dma_start(wo_t[:], wo16[:])

        # x^T fp16 [d-part, d-chunk, t], loaded per t-chunk
        x_t = persist.tile([128, 8 * T], F16, tag="xt", name="xt")
        xv = x_t[:].rearrange("p (c t) -> p c t", c=8)
        for tcc in range(NTC):
            tsl = slice(tcc * 512, (tcc + 1) * 512)
            nc.sync.dma_start(
                xv[:, :, tsl], xt16[:].rearrange("p (c t) -> p c t", c=8)[:, :, tsl]
            )

        qt = [persist.tile([128, T], F16, tag=f"qt{i}", name=f"qt{i}") for i in range(NF)]
        kt = [persist.tile([128, T], F16, tag=f"kt{i}", name=f"kt{i}") for i in range(NF)]
        vt = [persist.tile([128, 8 * 65], F16, tag=f"vt{i}", name=f"vt{i}") for i in range(NTT)]
        ao = [persist.tile([128, T], F16, tag=f"ao{i}", name=f"ao{i}") for i in range(NF)]
        for i in range(NTT):
            nc.sync.dma_start(vt[i][:, 64::65], ones8_t[:])

        wqv = w_ts["wq"][:].rearrange("p (c f) -> p c f", c=ND)
        wkv = w_ts["wk"][:].rearrange("p (c f) -> p c f", c=ND)
        wvv = w_ts["wv"][:].rearrange("p (c f) -> p c f", c=ND)

        def proj_qk_unit(tcc, fp, which):
            """Project+RoPE one [128f, 512t] tile of q or k."""
            tsl = slice(tcc * 512, (tcc + 1) * 512)
            wv_, dst = (wqv, qt) if which == "q" else (wkv, kt)
            fsl = slice(fp * 128, (fp + 1) * 128)
            pp = ppp.tile([128, 512], F32, tag="pp", name=f"p{which}{fp}c{tcc}")
            for c in range(ND):
                nc.tensor.matmul(
                    pp[:],
                    wv_[:, c, fsl],
                    xv[:, c, tsl],
                    start=(c == 0),
                    stop=(c == ND - 1),
                )
            raw = rope.tile([128, 512], F16, tag="raw")
            nc.vector.tensor_copy(raw[:], pp[:])
            rot = rope.tile([128, 512], F16, tag="rot")
            for hb in range(2):
                o = hb * 64
                nc.sync.dma_start(rot[o : o + 32, :], raw[o + 32 : o + 64, :])
                nc.sync.dma_start(rot[o + 32 : o + 64, :], raw[o : o + 32, :])
            d = dst[fp]
            nc.vector.tensor_mul(d[:, tsl], raw[:], cos_t[:, tsl])
            nc.vector.tensor_mul(rot[:], rot[:], sin_t[:, tsl])
            nc.vector.tensor_add(d[:, tsl], d[:, tsl], rot[:])

        def proj_v_unit(tcc, tb):
            """Project one [128t, 512f] tile of v into vt (heads-of-65 layout)."""
            i = tcc * 4 + tb
            tbs = slice(i * 128, (i + 1) * 128)
            pv = ppp.tile([128, 512], F32, tag="pp", name=f"pv{i}")
            for c in range(ND):
                nc.tensor.matmul(
                    pv[:],
                    xv[:, c, tbs],
                    wvv[:, c, :],
                    start=(c == 0),
                    stop=(c == ND - 1),
                )
            vdst = vt[i][:, :].rearrange("p (h c) -> p h c", c=65)[:, :, 0:64]
            vsrc = pv[:].rearrange("p (h c) -> p h c", c=64)
            nc.vector.tensor_copy(vdst, vsrc)

        def attn_unit(tcc, fp):
            """Attention for one f-tile (2 heads) over query chunk tcc."""
            t0 = tcc * 512
            ngrp = 2 * tcc + 2
            avs = []
            for ho in range(2):
                avs.append(pav.tile([128, 512], F32, tag="av", name=f"av{ho}"))
            for g2 in range(ngrp):
                exs = []
                for ho in range(2):
                    o = ho * 64
                    sc = psc.tile([128, 1024], F32, tag="sc", name="sc")
                    for half in range(2):
                        si = 2 * g2 + half
                        c0 = max(0, si * 128 - t0)
                        nc.tensor.matmul(
                            sc[:, half * 512 + c0 : half * 512 + 512],
                            kt[fp][o : o + 64, si * 128 : (si + 1) * 128],
                            qt[fp][o : o + 64, t0 + c0 : t0 + 512],
                            start=True,
                            stop=True,
                        )
                    ex = exp.tile([128, 1024], F16, tag="ex")
                    c0s = [max(0, (2 * g2 + h) * 128 - t0) for h in range(2)]
                    if c0s[0] == 0 and c0s[1] == 0:
                        nc.scalar.activation(
                            ex[:], sc[:], mybir.ActivationFunctionType.Exp,
                            scale=EXP_SCALE,
                        )
                    else:
                        for half in range(2):
                            lo = half * 512 + c0s[half]
                            hi = half * 512 + 512
                            nc.scalar.activation(
                                ex[:, lo:hi], sc[:, lo:hi],
                                mybir.ActivationFunctionType.Exp,
                                scale=EXP_SCALE,
                            )
                    for half in range(2):
                        si = 2 * g2 + half
                        c0 = si * 128 - t0
                        if c0 >= 0:
                            nc.vector.tensor_mul(
                                ex[:, half * 512 + c0 : half * 512 + c0 + 128],
                                ex[:, half * 512 + c0 : half * 512 + c0 + 128],
                                tri_t[:],
                            )
                    exs.append(ex)
                for ho in range(2):
                    h8 = fp * 2 + ho
                    for half in range(2):
                        si = 2 * g2 + half
                        c0 = max(0, si * 128 - t0)
                        nc.tensor.matmul(
                            avs[ho][0:65, c0:512],
                            vt[si][:, h8 * 65 : h8 * 65 + 65],
                            exs[ho][:, half * 512 + c0 : half * 512 + 512],
                            start=(g2 == 0 and half == 0),
                            stop=(g2 == ngrp - 1 and half == 1),
                        )
            for ho in range(2):
                o = ho * 64
                rcp = nrm.tile([1, 512], F16, tag="rcp")
                with nc.allow_low_precision(reason="softmax denom recip"):
                    nc.vector.reciprocal(rcp[:], avs[ho][64:65, :])
                # broadcast 1/denom into the unused upper rows of the av tile
                nc.tensor.matmul(
                    avs[ho][64:128, :], ones64_t[:], rcp[:], start=True, stop=True
                )
                # DVE may read only one PSUM operand: stage av rows in SBUF
                av16 = nrm.tile([64, 512], F16, tag="av16")
                nc.vector.tensor_copy(av16[:], avs[ho][0:64, :])
                nc.vector.tensor_mul(
                    ao[fp][o : o + 64, t0 : t0 + 512],
                    av16[:],
                    avs[ho][64:128, :],
                )

        def outproj_unit(i):
            """Output projection for one t-tile i: [128t, 1024]."""
            tbs = slice(i * 128, (i + 1) * 128)
            for n in range(2):
                po = ppp.tile([128, 512], F32, tag="pp", name=f"po{i}n{n}")
                for c in range(4):
                    nc.tensor.matmul(
                        po[:],
                        ao[c][:, tbs],
                        wo_t[:, c * D + n * 512 : c * D + n * 512 + 512],
                        start=(c == 0),
                        stop=(c == 3),
                    )
                oe = oev.tile([128, 512], F32, tag="oe")
                nc.vector.tensor_copy(oe[:], po[:])
                nc.sync.dma_start(out[tbs, n * 512 : (n + 1) * 512], oe[:])

        # ---- schedule: proj chunk 0 first, then attn(t) with proj(t+1) /
        # outproj of completed chunks spliced between attention blocks.
        for fp in range(NF):
            proj_qk_unit(0, fp, "q")
            proj_qk_unit(0, fp, "k")
        for tb in range(4):
            proj_v_unit(0, tb)

        for tcc in range(NTC):
            filler = []
            if tcc + 1 < NTC:
                nxt = tcc + 1
                for fp in range(NF):
                    filler.append(lambda fp=fp: proj_qk_unit(nxt, fp, "q"))
                    filler.append(lambda fp=fp: proj_qk_unit(nxt, fp, "k"))
                for tb in range(4):
                    filler.append(lambda tb=tb: proj_v_unit(nxt, tb))
            else:
                for i in range(12):
                    filler.append(lambda i=i: outproj_unit(i))
            nf_ = len(filler)
            done = 0
            for fp in range(NF):
                attn_unit(tcc, fp)
                take = nf_ * (fp + 1) // NF
                for j in range(done, take):
                    filler[j]()
                done = take
        for i in range(12, 16):
            outproj_unit(i)

    _split_waits(nc)
    return nc


_NC_CACHE = None


def _get_nc():
    global _NC_CACHE
    if _NC_CACHE is None:
        _NC_CACHE = _build_program()
    return _NC_CACHE


def _f16(a):
    return np.asarray(a, np.float32).astype(np.float16)


def _pack_x(xb):
    """x [T, D] -> [128, 8, T] fp16: out[p, c, t] = x[t, 128c+p]."""
    xt = xb.T.reshape(8, 128, T)
    return _f16(np.ascontiguousarray(xt.transpose(1, 0, 2).reshape(128, 8 * T)))


def _pack_w(w):
    """W slice [D, FC] -> [128, 8, FC] fp16."""
    wt = w.reshape(8, 128, FC)
    return _f16(np.ascontiguousarray(wt.transpose(1, 0, 2).reshape(128, 8 * FC)))


def _host_inputs(x, cos, sin, Wq, Wk, Wv, Wo):
    cos2 = _f16(np.ascontiguousarray(np.tile(cos.T, (2, 1))))
    sin2f = np.ascontiguousarray(np.tile(sin.T, (2, 1))).astype(np.float32)
    sin2f[0:32] *= -1.0
    sin2f[64:96] *= -1.0
    sin2 = _f16(sin2f)
    p = np.arange(128)
    tri_np = _f16((np.arange(128)[None, :] >= p[:, None]).astype(np.float32))
    ones8 = _f16(np.ones((128, 8), np.float32))

    in_maps = []
    for c in range(8):
        b, hh = c // 2, c % 2
        sl = slice(hh * FC, (hh + 1) * FC)
        wo = np.ascontiguousarray(Wo[sl, :]).reshape(4, 128, D)
        wo16 = _f16(np.ascontiguousarray(wo.transpose(1, 0, 2).reshape(128, 4 * D)))
        in_maps.append(
            {
                "xt16": _pack_x(np.ascontiguousarray(x[b])),
                "wq16": _pack_w(np.ascontiguousarray(Wq[:, sl])),
                "wk16": _pack_w(np.ascontiguousarray(Wk[:, sl])),
                "wv16": _pack_w(np.ascontiguousarray(Wv[:, sl])),
                "wo16": wo16,
                "cos2": cos2,
                "sin2": sin2,
                "tri": tri_np,
                "ones8": ones8,
                "ones64": _f16(np.ones((1, 64), np.float32)),
            }
        )
    return in_maps


def kernel(x, cos, sin, Wq, Wk, Wv, Wo):
    from concourse.bass_utils import run_bass_kernel_spmd

    x = np.asarray(x, dtype=np.float32)
    cos = np.asarray(cos, dtype=np.float32)
    sin = np.asarray(sin, dtype=np.float32)
    Wq = np.asarray(Wq, dtype=np.float32)
    Wk = np.asarray(Wk, dtype=np.float32)
    Wv = np.asarray(Wv, dtype=np.float32)
    Wo = np.asarray(Wo, dtype=np.float32)

    in_maps = _host_inputs(x, cos, sin, Wq, Wk, Wv, Wo)
    nc = _get_nc()
    res = run_bass_kernel_spmd(nc, in_maps, core_ids=list(range(8)))
    outs = [res.results[c]["out"] for c in range(8)]
    full = np.stack([outs[2 * b] + outs[2 * b + 1] for b in range(B)])
    return full.astype(np.float32)


# ---- simcheck support ----------------------------------------------------


def _core0_partial(np_inputs):
    x = np_inputs["x"][0].astype(np.float64)
    cos, sin = np_inputs["cos"].astype(np.float64), np_inputs["sin"].astype(np.float64)
    Wq, Wk, Wv, Wo = (np_inputs[k].astype(np.float64) for k in ("Wq", "Wk", "Wv", "Wo"))
    sl = slice(0, 512)
    q = (x @ Wq[:, sl]).reshape(T, 8, Dh)
    k = (x @ Wk[:, sl]).reshape(T, 8, Dh)
    v = (x @ Wv[:, sl]).reshape(T, 8, Dh)

    def rope_(z):
        half = Dh // 2
        rot = np.concatenate([-z[..., half:], z[..., :half]], axis=-1)
        return z * cos[:, None, :] + rot * sin[:, None, :]

    q, k = rope_(q), rope_(k)
    scores = np.einsum("thd,shd->hts", q, k) / np.sqrt(Dh)
    mask = np.tril(np.ones((T, T), dtype=bool))
    scores = np.where(mask[None], scores, -np.inf)
    w = np.exp(scores - scores.max(-1, keepdims=True))
    w = w / w.sum(-1, keepdims=True)
    o = np.einsum("hts,shd->thd", w, v).reshape(T, 512)
    return (o @ Wo[sl]).astype(np.float32)


def sim_inputs(np_inputs):
    in_maps = _host_inputs(
        np_inputs["x"], np_inputs["cos"], np_inputs["sin"],
        np_inputs["Wq"], np_inputs["Wk"], np_inputs["Wv"], np_inputs["Wo"],
    )
    return in_maps[0], _core0_partial(np_inputs)
